# revision 75
# baseline (speedup 1.0000x reference)
"""Trainium2 Bass kernel for the ULA beamformer DOA problem.

Contract: kernel(**inputs) takes FULL unsharded inputs (B=128 batch), shards
batch across 8 NeuronCores, runs a Bass/Tile kernel per core, and returns the
full (B, M) float32 angle labels.

Device algorithm per core (16 batch items):
  1. Load XP_b = [Xr_b; Xi_b] (128 x 1024), PE-transpose 128-col chunks.
  2. Gram G_b = Z^T Z (128 x 128) accumulated in PSUM over 8 chunks
     (all four real/imag covariance blocks at once), fp32r matmuls paired
     two batches wide (256-wide moving operand) for full PE rate.
  3. Toeplitz reduction: the ULA spectrum only needs the diagonal sums of the
     Hermitian covariance; cos/sin tables are the steering rows themselves.
     Diagonals are extracted with a stride-129 DRAM access pattern, masked and
     signed on DVE, column-summed with a ones-matmul -> W (128 x 16).
  4. spectrum = W^T @ SS_scaled  (one matmul, 16 x 3600).
  5. Peak detect (>= left, > right) + top-8 via DVE max/max_index.

Host: top-M selection from device top-8, with fp64 refinement of numerically
risky candidates (flat-top / near-tie cases) using the reference's direct
quadratic form.
"""

import numpy as np

B, N, T, A = 128, 64, 1024, 3600
NCORES = 8
BL = B // NCORES  # 16 batch items per core
BIG = np.float32(1e30)
RISK_RANK = 3e-5
RISK_FLAT = 2e-5

_cache = {}


def _host_constants():
    ident = np.eye(128, dtype=np.float32)
    masksgn = np.zeros((128, 128), np.float32)
    for p in range(128):
        n = p % 64
        for dp in range(128):
            d = dp % 64
            if n + d > 63:
                continue
            masksgn[p, dp] = 1.0 if dp < 64 else (-1.0 if p < 64 else 1.0)
    onescol = np.ones((128, 1), np.float32)
    scalevec = np.zeros((128, 1), np.float32)
    scalevec[0, 0] = 1.0 / T
    scalevec[1:64, 0] = 2.0 / T
    scalevec[64:, 0] = -2.0 / T
    return ident, masksgn, onescol, scalevec


def build_program(loop_n=None):
    """Build and compile the per-core Bass program. Returns the Bacc instance.
    loop_n wraps the whole body in an on-device loop (benchmarking only)."""
    key = ("nc", loop_n)
    if key in _cache:
        return _cache[key]
    from contextlib import ExitStack

    import concourse.bacc as bacc
    import concourse.mybir as mybir
    from concourse import tile
    from concourse.ap import AP

    f32 = mybir.dt.float32
    f32r = mybir.dt.float32r
    u32 = mybir.dt.uint32

    nc = bacc.Bacc("TRN2", target_bir_lowering=False, debug=False)

    xr_d = nc.dram_tensor("xr", (BL, N, T), f32, kind="ExternalInput")
    xi_d = nc.dram_tensor("xi", (BL, N, T), f32, kind="ExternalInput")
    sr_d = nc.dram_tensor("sr", (N, A), f32, kind="ExternalInput")
    si_d = nc.dram_tensor("si", (N, A), f32, kind="ExternalInput")
    ident_d = nc.dram_tensor("ident", (128, 128), f32, kind="ExternalInput")
    msk_d = nc.dram_tensor("masksgn", (128, 128), f32, kind="ExternalInput")
    ones_d = nc.dram_tensor("onescol", (128, 1), f32, kind="ExternalInput")
    scl_d = nc.dram_tensor("scalevec", (128, 1), f32, kind="ExternalInput")

    out_spec = nc.dram_tensor("out_spec", (BL, A), f32, kind="ExternalOutput")
    # top-8 per (eighth-chunk, batch) row; host merges the 8 chunks per batch
    out_idx = nc.dram_tensor("out_idx", (8 * BL, 8), u32, kind="ExternalOutput")
    out_val = nc.dram_tensor("out_val", (8 * BL, 8), f32, kind="ExternalOutput")

    g_dram = [nc.dram_tensor(f"gscr{b}", (130, 128), f32) for b in range(BL)]

    with tile.TileContext(nc) as tc, ExitStack() as ctx:
        const = ctx.enter_context(tc.tile_pool(name="const", bufs=1))
        xp_pool = ctx.enter_context(tc.tile_pool(name="xp", bufs=4))
        z_pool = ctx.enter_context(tc.tile_pool(name="z", bufs=2))
        g_pool = ctx.enter_context(tc.tile_pool(name="g", bufs=4))
        up_pool = ctx.enter_context(tc.tile_pool(name="up", bufs=4))
        w_pool = ctx.enter_context(tc.tile_pool(name="w", bufs=1))
        spec_pool = ctx.enter_context(tc.tile_pool(name="spec", bufs=1))
        pz = ctx.enter_context(tc.tile_pool(name="pz", bufs=4, space="PSUM"))
        pg = ctx.enter_context(tc.tile_pool(name="pg", bufs=2, space="PSUM"))
        ps = ctx.enter_context(tc.tile_pool(name="ps", bufs=2, space="PSUM"))

        # ---- input prefetch for pair 0 BEFORE the bulky constant loads so
        # the PE can start transposing as early as possible
        def load_pair_inputs(pair):
            b1 = 2 * pair
            # XPpair: [ Xr(b1) | Xr(b2) ] on partitions 0:64 (1024 cols each),
            # [ Xi(b1) | Xi(b2) ] on partitions 64:128. Per-b DMAs so the
            # first transposes can start as soon as b1's components land; for
            # pair 0, b1's components are further split so the very first
            # transpose group only waits for a quarter of the data.
            xpp = xp_pool.tile([128, 2 * T], f32, tag="xp", name=f"xpp{pair}")
            for h in (0, 1):
                cuts = (0, 512, T) if (pair == 0 and h == 0) else (0, T)
                for s, e in zip(cuts[:-1], cuts[1:]):
                    nc.sync.dma_start(
                        xpp[0:64, h * T + s:h * T + e], xr_d.ap()[b1 + h, :, s:e]
                    )
                    nc.sync.dma_start(
                        xpp[64:128, h * T + s:h * T + e], xi_d.ap()[b1 + h, :, s:e]
                    )
            return xpp

        ident_t = const.tile([128, 128], f32)
        nc.sync.dma_start(ident_t[:], ident_d.ap())
        xpp_next = load_pair_inputs(0) if loop_n is None else None

        msk_t = const.tile([128, 128], f32)
        nc.sync.dma_start(msk_t[:], msk_d.ap())
        ones_t = const.tile([128, 1], f32)
        nc.sync.dma_start(ones_t[:], ones_d.ap())
        scl_t = const.tile([128, 1], f32)
        nc.sync.dma_start(scl_t[:], scl_d.ap())

        # SS_scaled: rows 0..63 = steer_real * (2-d0)/T, rows 64.. = steer_imag
        # * -2/T, then split into fp32r hi/lo planes for the 1-cycle/row
        # spectrum matmuls. The big scale/split engine ops are emitted
        # mid-pipeline (see the pair loop) so they don't block the early
        # pairs' PSUM evacuations on the ACT queue.
        SS_t = const.tile([128, A], f32)
        SShi_t = const.tile([128, A], f32r)
        SSlo_t = const.tile([128, A], f32r)

        def emit_ss_load():
            # quarter-row chunks so no single transfer hogs the DMA engines
            for lo in range(0, 64, 16):
                nc.gpsimd.dma_start(SS_t[lo:lo + 16, :], sr_d.ap()[lo:lo + 16])
                nc.gpsimd.dma_start(
                    SS_t[64 + lo:80 + lo, :], si_d.ap()[lo:lo + 16]
                )

        def emit_ss_prep():
            nc.gpsimd.tensor_scalar_mul(SS_t[:], SS_t[:], scl_t[:, 0:1])
            nc.scalar.copy(SShi_t[:], SS_t[:])
            nc.vector.tensor_tensor(
                SSlo_t[:], SS_t[:], SShi_t[:], op=mybir.AluOpType.subtract
            )

        # W columns per half-batch; separate tiles so the first half's
        # spectrum/peaks can run while pairs 4..7 are still computing
        W_ts = [
            w_pool.tile([128, BL // 2], f32, name=f"W{h}") for h in range(2)
        ]

        # pad gscratch rows 128:130 once with finite junk, off the per-b
        # critical chain (values masked out later; must just not be NaN)
        for b in range(BL):
            nc.gpsimd.dma_start(g_dram[b].ap()[128:130, :], ident_t[0:2, :])

        # peak-mask tiles; non-peaks and the border columns become 0.0, which
        # ranks below every real peak (spectrum values here are all >> 0)
        mskf_ts = [spec_pool.tile([64, 452], f32, name=f"mskf{h}") for h in range(2)]
        nc.gpsimd.memset(mskf_ts[0][:], 0.0)
        nc.gpsimd.memset(mskf_ts[1][:], 0.0)

        def do_spectrum_half(half):
            # spectrum rows for b in [half*8, half*8+8). Reshaped on the fly
            # to (64, 452) with one-column halos: peak detection then uses 64
            # partitions instead of 8 (8x fewer DVE cycles). Eighth j, local
            # column l <-> global angle a = 450*j - 1 + l. Border pads are
            # written first (+BIG so a=0 / a=A-1 never count as peaks); the
            # halo DMAs overwrite the pad cells of interior eighths.
            spec_t = spec_pool.tile([BL // 2, A], f32, name=f"spec{half}")
            sp4 = spec_pool.tile([64, 452], f32, name=f"sp4_{half}")
            nc.gpsimd.memset(sp4[0:64, 0:1], float(BIG))
            nc.gpsimd.memset(sp4[0:64, 451:452], float(BIG))

            # half 0 overlaps pairs 5..7, so its reshapes ride the idle Pool
            # (SWDGE) queue to keep the sync queue free for the pair DMAs;
            # half 1 is the tail and wants the faster HWDGE path
            eng = nc.gpsimd if half == 0 else nc.sync

            def mk_reshape(j):
                if j == 0:
                    return lambda: eng.dma_start(
                        sp4[0:8, 1:452], spec_t[:, 0:451]
                    )
                if j == 7:
                    return lambda: eng.dma_start(
                        sp4[56:64, 0:451], spec_t[:, 3149:3600]
                    )
                return lambda: eng.dma_start(
                    sp4[8 * j:8 * j + 8, 0:452],
                    spec_t[:, 450 * j - 1:450 * j + 451],
                )

            # eighth j is complete once spectrum columns < 450*j + 451 exist;
            # with 512-wide chunks that is exactly after chunk j
            reshape = {512 * (j + 1): mk_reshape(j) for j in range(7)}
            reshape[3600] = mk_reshape(7)
            # W hi/lo planes for the fp32r spectrum matmuls
            whi = spec_pool.tile([128, BL // 2], f32r, name=f"whi{half}")
            wlo = spec_pool.tile([128, BL // 2], f32r, name=f"wlo{half}")
            nc.scalar.copy(whi[:], W_ts[half][:])
            nc.vector.tensor_tensor(
                wlo[:], W_ts[half][:], whi[:], op=mybir.AluOpType.subtract
            )
            for off in range(0, A, 512):
                cw = min(512, A - off)
                pst = ps.tile([BL // 2, cw], f32, tag="ps", name=f"ps{half}_{off}")
                nc.tensor.matmul(pst[:], whi[:], SShi_t[:, off:off + cw],
                                 start=True, stop=False)
                nc.tensor.matmul(pst[:], whi[:], SSlo_t[:, off:off + cw],
                                 start=False, stop=False)
                nc.tensor.matmul(pst[:], wlo[:], SShi_t[:, off:off + cw],
                                 start=False, stop=True)
                nc.scalar.copy(spec_t[:, off:off + cw], pst[:])
                if off + cw in reshape:
                    reshape[off + cw]()

            nc.sync.dma_start(
                out_spec.ap()[half * (BL // 2):(half + 1) * (BL // 2), :], spec_t[:]
            )
            return sp4

        def do_peaks_half(half, sp4):
            # at-least-left, strictly-above-right: flat tops keep their
            # rightmost member so near-equal peaks are not annihilated
            m1u = spec_pool.tile([64, 450], f32, name=f"m1u{half}")
            m2u = spec_pool.tile([64, 450], f32, name=f"m2u{half}")
            nc.vector.tensor_tensor(
                m1u[:], sp4[:, 1:451], sp4[:, 0:450], op=mybir.AluOpType.is_ge
            )
            nc.vector.tensor_tensor(
                m2u[:], sp4[:, 1:451], sp4[:, 2:452], op=mybir.AluOpType.is_gt
            )
            nc.vector.tensor_mul(m1u[:], m1u[:], m2u[:])
            mskf = mskf_ts[half]
            nc.vector.tensor_mul(mskf[:, 1:451], m1u[:], sp4[:, 1:451])
            val8_t = spec_pool.tile([64, 8], f32, name=f"val8_{half}")
            idx8_t = spec_pool.tile([64, 8], u32, name=f"idx8_{half}")
            nc.vector.max(val8_t[:], mskf[:])
            nc.vector.max_index(idx8_t[:], val8_t[:], mskf[:])
            nc.sync.dma_start(out_idx.ap()[64 * half:64 * (half + 1), :], idx8_t[:])
            nc.sync.dma_start(out_val.ap()[64 * half:64 * (half + 1), :], val8_t[:])

        def emit_transposes(pair, xpp):
            # Zbig layout: [b1 chunks 0..7 | b2 chunks 0..7], 1024 cols each,
            # split into fp32r hi/lo planes: G = Zhi^T Zhi + Zhi^T Zlo +
            # Zlo^T Zhi runs the PE at 1 cycle/row (vs 4 for fp32) with
            # ~2^-18 product precision. Transposes pack 4 chunks into one
            # PSUM bank tile; the hi plane is the (rounding) PSUM evacuation
            # copy, the lo plane is one extra subtract.
            zhi = z_pool.tile([128, 2048], f32r, tag="zhi", name=f"zhi{pair}")
            zlo = z_pool.tile([128, 2048], f32r, tag="zlo", name=f"zlo{pair}")
            for q in (0, 1):
                for h in (0, 1):
                    pzt = pz.tile([128, 512], f32, tag="pz", name=f"pz{pair}{q}{h}")
                    for k in range(4):
                        c = 4 * q + k
                        nc.tensor.matmul(
                            pzt[:, k * 128:(k + 1) * 128],
                            xpp[:, h * T + c * 128:h * T + (c + 1) * 128],
                            ident_t[:],
                            is_transpose=True,
                            start=(k == 0),
                            stop=(k == 3),
                        )
                    lo = h * 1024 + q * 512
                    nc.scalar.copy(zhi[:, lo:lo + 512], pzt[:])
                    nc.vector.tensor_tensor(
                        zlo[:, lo:lo + 512], pzt[:], zhi[:, lo:lo + 512],
                        op=mybir.AluOpType.subtract,
                    )
            return zhi, zlo

        gts = {}
        upts = {}

        def emit_grams(pair, zhi, zlo):
            zhi3 = zhi[:].rearrange("p (h c) -> p h c", h=2)
            zlo3 = zlo[:].rearrange("p (h c) -> p h c", h=2)
            # both G halves live in one PSUM bank as a single accumulation
            # group: [G(b1) cols 0:256 | G(b2) cols 256:512]; the spare
            # 128-col block of each half later holds that b's column-sum
            gt = pg.tile([128, 512], f32, tag="gt", name=f"gt{pair}")
            gts[pair] = gt
            n_mm = 0
            for c in range(8):
                rhs_hi = zhi3[:, :, c * 128:(c + 1) * 128]
                rhs_lo = zlo3[:, :, c * 128:(c + 1) * 128]
                for h in (0, 1):
                    lo = h * 1024 + c * 128
                    for lh, rh in (
                        (zhi[:, lo:lo + 128], rhs_hi),
                        (zhi[:, lo:lo + 128], rhs_lo),
                        (zlo[:, lo:lo + 128], rhs_hi),
                    ):
                        nc.tensor.matmul(
                            gt[:, h * 256:(h + 1) * 256],
                            lh,
                            rh,
                            start=(n_mm == 0),
                            stop=(n_mm == 47),
                        )
                        n_mm += 1

        def emit_diag_start(pair):
            # G -> DRAM -> stride-129 diagonal reload -> signed mask; no PE
            # work, so it can chase the grams immediately
            gt = gts[pair]
            for h, b in ((0, 2 * pair), (1, 2 * pair + 1)):
                gd = g_dram[b]
                gsb = g_pool.tile([128, 128], f32, tag="g", name=f"gsb{b}")
                nc.scalar.copy(gsb[:], gt[:, h * 256 + h * 128:h * 256 + (h + 1) * 128])
                nc.sync.dma_start(gd.ap()[0:128, :], gsb[:])
                upt = up_pool.tile([128, 128], f32, tag="up", name=f"up{b}")
                upts[b] = upt
                nc.sync.dma_start(upt[:], AP(gd, 0, [[129, 128], [1, 128]]))
                nc.sync.dma_start(
                    upt[64:128, 64:128], AP(gd, 64 * 128, [[129, 64], [1, 64]])
                )
                nc.vector.tensor_mul(upt[:], upt[:], msk_t[:])

        def emit_diag_finish(pair):
            # column-sums (PE) one pipeline stage later, when the DRAM
            # round-trip has surely completed
            gt = gts.pop(pair)
            for h, b in ((0, 2 * pair), (1, 2 * pair + 1)):
                upt = upts.pop(b)
                wcol = gt[:, h * 256 + (1 - h) * 128:h * 256 + (1 - h) * 128 + 1]
                nc.tensor.matmul(wcol, upt[:], ones_t[:])
                nc.scalar.copy(W_ts[b // 8][:, (b % 8):(b % 8) + 1], wcol)

        # software pipeline: transposes of pair p run on the PE while the
        # grams of pair p-1 stream and the diagonal extraction of pair p-2
        # finishes, hiding both the PSUM-evacuation (copy+subtract) latency
        # and the DRAM diagonal round-trip
        def emit_body(inline_ss, first_xpp=None):
            zq_prev = emit_transposes(0, first_xpp or load_pair_inputs(0))
            sp4_lo = None
            for pair in range(1, BL // 2):
                xpp = load_pair_inputs(pair)
                zq = emit_transposes(pair, xpp)
                emit_grams(pair - 1, *zq_prev)
                emit_diag_start(pair - 1)
                if pair >= 2:
                    emit_diag_finish(pair - 2)
                zq_prev = zq
                if inline_ss and pair == 3:
                    emit_ss_load()
                if inline_ss and pair == 4:
                    emit_ss_prep()
                if pair - 2 == 3:
                    sp4_lo = do_spectrum_half(0)
                if pair - 2 == 4:
                    do_peaks_half(0, sp4_lo)
            last = BL // 2 - 1
            emit_grams(last, *zq_prev)
            emit_diag_start(last)
            emit_diag_finish(last - 1)
            emit_diag_finish(last)
            sp4_hi = do_spectrum_half(1)
            do_peaks_half(1, sp4_hi)

        if loop_n is None:
            emit_body(inline_ss=True, first_xpp=xpp_next)
        else:
            emit_ss_load()
            emit_ss_prep()
            with tc.For_i(0, loop_n, 1):
                emit_body(inline_ss=False)

    nc.compile()
    _cache[key] = nc
    return nc


def _is_ula(sr, si, atol=1e-3):
    """Check the steering matrix has the phase-additive ULA structure the
    Toeplitz reduction relies on."""
    if not (np.allclose(sr[0], 1.0, atol=atol) and np.allclose(si[0], 0.0, atol=atol)):
        return False
    # conj(S_n) * S_{n+1} should equal S_1 for every n
    re = sr[:-1] * sr[1:] + si[:-1] * si[1:]
    im = sr[:-1] * si[1:] - si[:-1] * sr[1:]
    return bool(
        np.allclose(re, sr[1][None, :], atol=atol)
        and np.allclose(im, si[1][None, :], atol=atol)
    )


def _fallback_numpy(x_real, x_imag, steer_real, steer_imag, angles, M):
    x = x_real.astype(np.float32) + 1j * x_imag.astype(np.float32)
    cov = np.matmul(x, np.conj(np.swapaxes(x, 1, 2))) / np.float32(T)
    S = steer_real.astype(np.float32) + 1j * steer_imag.astype(np.float32)
    spec = np.einsum("na,bnm,ma->ba", np.conj(S), cov, S).real.astype(np.float32)
    labels = np.zeros((spec.shape[0], M), np.float32)
    for b in range(spec.shape[0]):
        s = spec[b]
        pk = (s[1:-1] > s[:-2]) & (s[1:-1] > s[2:])
        masked = np.full(A, -np.inf, np.float32)
        masked[1:-1][pk] = s[1:-1][pk]
        order = np.argsort(-masked, kind="stable")[:M]
        labels[b] = angles[order]
    return labels


def _select_labels(spec, idx8, val8, x_real, x_imag, steer_real, steer_imag,
                   angles, M):
    """Top-M selection from device top-8 candidates with fp64 refinement of
    numerically risky (near-tie / flat-top) cases."""
    S64 = steer_real.astype(np.float64) + 1j * steer_imag.astype(np.float64)
    labels = np.zeros((B, M), np.float32)
    for b in range(B):
        cands = idx8[b].astype(np.int64)
        vals = val8[b].astype(np.float64)
        s = spec[b]
        suspect = np.zeros(8, bool)
        for j in range(7):
            if vals[j + 1] > -1e29 and (vals[j] - vals[j + 1]) < RISK_RANK * abs(vals[j]):
                suspect[j] = suspect[j + 1] = True
        flat = np.zeros(8, bool)
        for j, c in enumerate(cands):
            if 1 <= c <= A - 2 and (
                abs(s[c] - s[c - 1]) < RISK_FLAT * abs(s[c])
                or abs(s[c] - s[c + 1]) < RISK_FLAT * abs(s[c])
            ):
                suspect[j] = flat[j] = True
        if not suspect.any():
            labels[b] = angles[cands[:M]]
            continue
        # fp64 evaluation of the reference's direct quadratic form at the
        # union of suspect windows
        bins = set()
        for j in range(8):
            if flat[j]:
                for o in range(-3, 4):
                    if 0 <= cands[j] + o < A:
                        bins.add(int(cands[j] + o))
            elif suspect[j]:
                bins.add(int(cands[j]))
        bins = sorted(bins)
        x64 = x_real[b].astype(np.float64) + 1j * x_imag[b].astype(np.float64)
        Y = np.conj(x64).T @ S64[:, bins]  # (T, len(bins))
        sv = dict(zip(bins, (np.abs(Y) ** 2).sum(axis=0) / T))
        # refined candidate list: (value, device_rank, position)
        refined = []
        for j in range(8):
            c = int(cands[j])
            if vals[j] < -1e29:
                continue
            if flat[j]:
                # true local-max position near c per fp64
                best = None
                for o in range(-2, 3):
                    a = c + o
                    if a - 1 in sv and a + 1 in sv and a in sv:
                        if sv[a] > sv[a - 1] and sv[a] > sv[a + 1]:
                            if best is None or sv[a] > sv[best]:
                                best = a
                if best is None:
                    best = c
                refined.append((float(sv[best]), best))
            elif suspect[j]:
                refined.append((float(sv[c]), c))
            else:
                refined.append((float(vals[j]), c))
        # dedupe positions (two flat candidates can refine to the same bin)
        seen = {}
        for v, p in refined:
            if p not in seen or v > seen[p]:
                seen[p] = v
        order = sorted(seen.items(), key=lambda kv: (-kv[1], kv[0]))
        sel = [p for p, _ in order[:M]]
        while len(sel) < M:
            for c in cands:
                if int(c) not in sel:
                    sel.append(int(c))
                    break
        labels[b] = angles[sel]
    return labels


def kernel(x_real, x_imag, steer_real, steer_imag, angles, M):
    x_real = np.ascontiguousarray(np.asarray(x_real), dtype=np.float32)
    x_imag = np.ascontiguousarray(np.asarray(x_imag), dtype=np.float32)
    steer_real = np.ascontiguousarray(np.asarray(steer_real), dtype=np.float32)
    steer_imag = np.ascontiguousarray(np.asarray(steer_imag), dtype=np.float32)
    angles = np.asarray(angles)
    M = int(M)

    if (
        x_real.shape != (B, N, T)
        or steer_real.shape != (N, A)
        or M > 8
        or not _is_ula(steer_real, steer_imag)
    ):
        return _fallback_numpy(x_real, x_imag, steer_real, steer_imag, angles, M)

    from concourse.bass_utils import run_bass_kernel_spmd

    nc = build_program()
    ident, masksgn, onescol, scalevec = _host_constants()
    in_maps = []
    for c in range(NCORES):
        sl = slice(c * BL, (c + 1) * BL)
        in_maps.append({
            "xr": x_real[sl],
            "xi": x_imag[sl],
            "sr": steer_real,
            "si": steer_imag,
            "ident": ident,
            "masksgn": masksgn,
            "onescol": onescol,
            "scalevec": scalevec,
        })
    res = run_bass_kernel_spmd(nc, in_maps, list(range(NCORES))).results

    spec = np.concatenate([res[c]["out_spec"] for c in range(NCORES)], axis=0)
    idx8, val8 = _merge_quarters(
        [res[c]["out_idx"] for c in range(NCORES)],
        [res[c]["out_val"] for c in range(NCORES)],
    )

    return _select_labels(
        spec, idx8, val8, x_real, x_imag, steer_real, steer_imag, angles, M
    )


def _merge_quarters(idx_list, val_list):
    """Merge per-(eighth-chunk, batch) top-8 rows into per-batch global
    top-8. Device row r of a core: half h = r // 64, chunk j = (r % 64) // 8,
    local batch = 8*h + (r % 8); local column l maps to global angle index
    450*j - 1 + l. Non-peak filler entries are 0.0."""
    ncores = len(idx_list)
    idx8 = np.zeros((ncores * BL, 8), np.int64)
    val8 = np.full((ncores * BL, 8), -np.float64(BIG), np.float32)
    for c in range(ncores):
        iv = idx_list[c].astype(np.int64)  # (8*BL, 8)
        vv = val_list[c]
        for bl in range(BL):
            h, bi = bl // 8, bl % 8
            cand_v = []
            cand_i = []
            for j in range(8):
                r = 64 * h + 8 * j + bi
                gi = 450 * j - 1 + iv[r]
                keep = vv[r] > 0.5
                cand_v.append(vv[r][keep])
                cand_i.append(gi[keep])
            cv = np.concatenate(cand_v)
            ci = np.concatenate(cand_i)
            order = np.lexsort((ci, -cv.astype(np.float64)))[:8]
            b = c * BL + bl
            val8[b, :len(order)] = cv[order]
            idx8[b, :len(order)] = ci[order]
    return idx8, val8


# revision 78
# speedup vs baseline: 1.0108x; 1.0108x over previous
"""Trainium2 Bass kernel for the ULA beamformer DOA problem.

Contract: kernel(**inputs) takes FULL unsharded inputs (B=128 batch), shards
batch across 8 NeuronCores, runs a Bass/Tile kernel per core, and returns the
full (B, M) float32 angle labels.

Device algorithm per core (16 batch items):
  1. Load XP_b = [Xr_b; Xi_b] (128 x 1024), PE-transpose 128-col chunks.
  2. Gram G_b = Z^T Z (128 x 128) accumulated in PSUM over 8 chunks
     (all four real/imag covariance blocks at once), fp32r matmuls paired
     two batches wide (256-wide moving operand) for full PE rate.
  3. Toeplitz reduction: the ULA spectrum only needs the diagonal sums of the
     Hermitian covariance; cos/sin tables are the steering rows themselves.
     Diagonals are extracted with a stride-129 DRAM access pattern, masked and
     signed on DVE, column-summed with a ones-matmul -> W (128 x 16).
  4. spectrum = W^T @ SS_scaled  (one matmul, 16 x 3600).
  5. Peak detect (>= left, > right) + top-8 via DVE max/max_index.

Host: top-M selection from device top-8, with fp64 refinement of numerically
risky candidates (flat-top / near-tie cases) using the reference's direct
quadratic form.
"""

import numpy as np

B, N, T, A = 128, 64, 1024, 3600
NCORES = 8
BL = B // NCORES  # 16 batch items per core
BIG = np.float32(1e30)
RISK_RANK = 3e-5
RISK_FLAT = 2e-5

_cache = {}


def _host_constants():
    ident = np.eye(128, dtype=np.float32)
    masksgn = np.zeros((128, 128), np.float32)
    for p in range(128):
        n = p % 64
        for dp in range(128):
            d = dp % 64
            if n + d > 63:
                continue
            masksgn[p, dp] = 1.0 if dp < 64 else (-1.0 if p < 64 else 1.0)
    onescol = np.ones((128, 1), np.float32)
    scalevec = np.zeros((128, 1), np.float32)
    scalevec[0, 0] = 1.0 / T
    scalevec[1:64, 0] = 2.0 / T
    scalevec[64:, 0] = -2.0 / T
    return ident, masksgn, onescol, scalevec


def build_program(loop_n=None):
    """Build and compile the per-core Bass program. Returns the Bacc instance.
    loop_n wraps the whole body in an on-device loop (benchmarking only)."""
    key = ("nc", loop_n)
    if key in _cache:
        return _cache[key]
    from contextlib import ExitStack

    import concourse.bacc as bacc
    import concourse.mybir as mybir
    from concourse import tile
    from concourse.ap import AP

    f32 = mybir.dt.float32
    f32r = mybir.dt.float32r
    u32 = mybir.dt.uint32

    nc = bacc.Bacc("TRN2", target_bir_lowering=False, debug=False)

    xr_d = nc.dram_tensor("xr", (BL, N, T), f32, kind="ExternalInput")
    xi_d = nc.dram_tensor("xi", (BL, N, T), f32, kind="ExternalInput")
    sr_d = nc.dram_tensor("sr", (N, A), f32, kind="ExternalInput")
    si_d = nc.dram_tensor("si", (N, A), f32, kind="ExternalInput")
    ident_d = nc.dram_tensor("ident", (128, 128), f32, kind="ExternalInput")
    msk_d = nc.dram_tensor("masksgn", (128, 128), f32, kind="ExternalInput")
    ones_d = nc.dram_tensor("onescol", (128, 1), f32, kind="ExternalInput")
    scl_d = nc.dram_tensor("scalevec", (128, 1), f32, kind="ExternalInput")

    out_spec = nc.dram_tensor("out_spec", (BL, A), f32, kind="ExternalOutput")
    # top-8 per (eighth-chunk, batch) row; host merges the 8 chunks per batch
    out_idx = nc.dram_tensor("out_idx", (8 * BL, 8), u32, kind="ExternalOutput")
    out_val = nc.dram_tensor("out_val", (8 * BL, 8), f32, kind="ExternalOutput")

    g_dram = [nc.dram_tensor(f"gscr{b}", (130, 128), f32) for b in range(BL)]

    with tile.TileContext(nc) as tc, ExitStack() as ctx:
        const = ctx.enter_context(tc.tile_pool(name="const", bufs=1))
        xp_pool = ctx.enter_context(tc.tile_pool(name="xp", bufs=4))
        z_pool = ctx.enter_context(tc.tile_pool(name="z", bufs=2))
        g_pool = ctx.enter_context(tc.tile_pool(name="g", bufs=4))
        up_pool = ctx.enter_context(tc.tile_pool(name="up", bufs=4))
        w_pool = ctx.enter_context(tc.tile_pool(name="w", bufs=1))
        spec_pool = ctx.enter_context(tc.tile_pool(name="spec", bufs=1))
        pz = ctx.enter_context(tc.tile_pool(name="pz", bufs=3, space="PSUM"))
        pg = ctx.enter_context(tc.tile_pool(name="pg", bufs=3, space="PSUM"))
        ps = ctx.enter_context(tc.tile_pool(name="ps", bufs=2, space="PSUM"))

        # ---- input prefetch for pair 0 BEFORE the bulky constant loads so
        # the PE can start transposing as early as possible
        def load_pair_inputs(pair):
            b1 = 2 * pair
            # XPpair: [ Xr(b1) | Xr(b2) ] on partitions 0:64 (1024 cols each),
            # [ Xi(b1) | Xi(b2) ] on partitions 64:128. Per-b DMAs so the
            # first transposes can start as soon as b1's components land; for
            # pair 0, b1's components are further split so the very first
            # transpose group only waits for a quarter of the data.
            xpp = xp_pool.tile([128, 2 * T], f32, tag="xp", name=f"xpp{pair}")
            for h in (0, 1):
                cuts = (0, 512, T) if (pair == 0 and h == 0) else (0, T)
                for s, e in zip(cuts[:-1], cuts[1:]):
                    nc.sync.dma_start(
                        xpp[0:64, h * T + s:h * T + e], xr_d.ap()[b1 + h, :, s:e]
                    )
                    nc.sync.dma_start(
                        xpp[64:128, h * T + s:h * T + e], xi_d.ap()[b1 + h, :, s:e]
                    )
            return xpp

        ident_t = const.tile([128, 128], f32)
        nc.sync.dma_start(ident_t[:], ident_d.ap())
        xpp_next = load_pair_inputs(0) if loop_n is None else None

        msk_t = const.tile([128, 128], f32)
        nc.sync.dma_start(msk_t[:], msk_d.ap())
        ones_t = const.tile([128, 1], f32)
        nc.sync.dma_start(ones_t[:], ones_d.ap())
        scl_t = const.tile([128, 1], f32)
        nc.sync.dma_start(scl_t[:], scl_d.ap())

        # SS_scaled: rows 0..63 = steer_real * (2-d0)/T, rows 64.. = steer_imag
        # * -2/T, then split into fp32r hi/lo planes for the 1-cycle/row
        # spectrum matmuls. The big scale/split engine ops are emitted
        # mid-pipeline (see the pair loop) so they don't block the early
        # pairs' PSUM evacuations on the ACT queue.
        SS_t = const.tile([128, A], f32)
        SShi_t = const.tile([128, A], f32r)
        SSlo_t = const.tile([128, A], f32r)

        def emit_ss_load():
            # quarter-row chunks so no single transfer hogs the DMA engines
            for lo in range(0, 64, 16):
                nc.gpsimd.dma_start(SS_t[lo:lo + 16, :], sr_d.ap()[lo:lo + 16])
                nc.gpsimd.dma_start(
                    SS_t[64 + lo:80 + lo, :], si_d.ap()[lo:lo + 16]
                )

        def emit_ss_prep():
            nc.gpsimd.tensor_scalar_mul(SS_t[:], SS_t[:], scl_t[:, 0:1])
            nc.scalar.copy(SShi_t[:], SS_t[:])
            nc.vector.tensor_tensor(
                SSlo_t[:], SS_t[:], SShi_t[:], op=mybir.AluOpType.subtract
            )

        # W columns per half-batch; separate tiles so the first half's
        # spectrum/peaks can run while pairs 4..7 are still computing
        W_ts = [
            w_pool.tile([128, BL // 2], f32, name=f"W{h}") for h in range(2)
        ]

        # pad gscratch rows 128:130 once with finite junk, off the per-b
        # critical chain (values masked out later; must just not be NaN)
        for b in range(BL):
            nc.gpsimd.dma_start(g_dram[b].ap()[128:130, :], ident_t[0:2, :])

        # peak-mask tiles; non-peaks and the border columns become 0.0, which
        # ranks below every real peak (spectrum values here are all >> 0)
        mskf_ts = [spec_pool.tile([64, 452], f32, name=f"mskf{h}") for h in range(2)]
        nc.gpsimd.memset(mskf_ts[0][:], 0.0)
        nc.gpsimd.memset(mskf_ts[1][:], 0.0)

        def do_spectrum_half(half):
            # spectrum rows for b in [half*8, half*8+8). Reshaped on the fly
            # to (64, 452) with one-column halos: peak detection then uses 64
            # partitions instead of 8 (8x fewer DVE cycles). Eighth j, local
            # column l <-> global angle a = 450*j - 1 + l. Border pads are
            # written first (+BIG so a=0 / a=A-1 never count as peaks); the
            # halo DMAs overwrite the pad cells of interior eighths.
            spec_t = spec_pool.tile([BL // 2, A], f32, name=f"spec{half}")
            sp4 = spec_pool.tile([64, 452], f32, name=f"sp4_{half}")
            nc.gpsimd.memset(sp4[0:64, 0:1], float(BIG))
            nc.gpsimd.memset(sp4[0:64, 451:452], float(BIG))

            # half 0 overlaps pairs 5..7, so its reshapes ride the idle Pool
            # (SWDGE) queue to keep the sync queue free for the pair DMAs;
            # half 1 is the tail and wants the faster HWDGE path
            eng = nc.gpsimd if half == 0 else nc.sync

            def mk_reshape(j):
                if j == 0:
                    return lambda: eng.dma_start(
                        sp4[0:8, 1:452], spec_t[:, 0:451]
                    )
                if j == 7:
                    return lambda: eng.dma_start(
                        sp4[56:64, 0:451], spec_t[:, 3149:3600]
                    )
                return lambda: eng.dma_start(
                    sp4[8 * j:8 * j + 8, 0:452],
                    spec_t[:, 450 * j - 1:450 * j + 451],
                )

            # eighth j is complete once spectrum columns < 450*j + 451 exist;
            # with 512-wide chunks that is exactly after chunk j
            reshape = {512 * (j + 1): mk_reshape(j) for j in range(7)}
            reshape[3600] = mk_reshape(7)
            # W hi/lo planes for the fp32r spectrum matmuls
            whi = spec_pool.tile([128, BL // 2], f32r, name=f"whi{half}")
            wlo = spec_pool.tile([128, BL // 2], f32r, name=f"wlo{half}")
            nc.scalar.copy(whi[:], W_ts[half][:])
            nc.vector.tensor_tensor(
                wlo[:], W_ts[half][:], whi[:], op=mybir.AluOpType.subtract
            )
            for off in range(0, A, 512):
                cw = min(512, A - off)
                pst = ps.tile([BL // 2, cw], f32, tag="ps", name=f"ps{half}_{off}")
                nc.tensor.matmul(pst[:], whi[:], SShi_t[:, off:off + cw],
                                 start=True, stop=False)
                nc.tensor.matmul(pst[:], whi[:], SSlo_t[:, off:off + cw],
                                 start=False, stop=False)
                nc.tensor.matmul(pst[:], wlo[:], SShi_t[:, off:off + cw],
                                 start=False, stop=True)
                nc.scalar.copy(spec_t[:, off:off + cw], pst[:])
                if off + cw in reshape:
                    reshape[off + cw]()

            nc.sync.dma_start(
                out_spec.ap()[half * (BL // 2):(half + 1) * (BL // 2), :], spec_t[:]
            )
            return sp4

        def do_peaks_half(half, sp4):
            # at-least-left, strictly-above-right: flat tops keep their
            # rightmost member so near-equal peaks are not annihilated
            m1u = spec_pool.tile([64, 450], f32, name=f"m1u{half}")
            m2u = spec_pool.tile([64, 450], f32, name=f"m2u{half}")
            nc.vector.tensor_tensor(
                m1u[:], sp4[:, 1:451], sp4[:, 0:450], op=mybir.AluOpType.is_ge
            )
            nc.vector.tensor_tensor(
                m2u[:], sp4[:, 1:451], sp4[:, 2:452], op=mybir.AluOpType.is_gt
            )
            nc.vector.tensor_mul(m1u[:], m1u[:], m2u[:])
            mskf = mskf_ts[half]
            nc.vector.tensor_mul(mskf[:, 1:451], m1u[:], sp4[:, 1:451])
            val8_t = spec_pool.tile([64, 8], f32, name=f"val8_{half}")
            idx8_t = spec_pool.tile([64, 8], u32, name=f"idx8_{half}")
            nc.vector.max(val8_t[:], mskf[:])
            nc.vector.max_index(idx8_t[:], val8_t[:], mskf[:])
            nc.sync.dma_start(out_idx.ap()[64 * half:64 * (half + 1), :], idx8_t[:])
            nc.sync.dma_start(out_val.ap()[64 * half:64 * (half + 1), :], val8_t[:])

        def emit_transposes(pair, xpp):
            # Zbig layout: [b1 chunks 0..7 | b2 chunks 0..7], 1024 cols each,
            # split into fp32r hi/lo planes: G = Zhi^T Zhi + Zhi^T Zlo +
            # Zlo^T Zhi runs the PE at 1 cycle/row (vs 4 for fp32) with
            # ~2^-18 product precision. Transposes pack 4 chunks into one
            # PSUM bank tile; the hi plane is the (rounding) PSUM evacuation
            # copy, the lo plane is one extra subtract.
            zhi = z_pool.tile([128, 2048], f32r, tag="zhi", name=f"zhi{pair}")
            zlo = z_pool.tile([128, 2048], f32r, tag="zlo", name=f"zlo{pair}")
            for q in (0, 1):
                for h in (0, 1):
                    pzt = pz.tile([128, 512], f32, tag="pz", name=f"pz{pair}{q}{h}")
                    for k in range(4):
                        c = 4 * q + k
                        nc.tensor.matmul(
                            pzt[:, k * 128:(k + 1) * 128],
                            xpp[:, h * T + c * 128:h * T + (c + 1) * 128],
                            ident_t[:],
                            is_transpose=True,
                            start=(k == 0),
                            stop=(k == 3),
                        )
                    lo = h * 1024 + q * 512
                    nc.scalar.copy(zhi[:, lo:lo + 512], pzt[:])
                    nc.vector.tensor_tensor(
                        zlo[:, lo:lo + 512], pzt[:], zhi[:, lo:lo + 512],
                        op=mybir.AluOpType.subtract,
                    )
            return zhi, zlo

        gts = {}
        upts = {}

        def emit_grams(pair, zhi, zlo):
            zhi3 = zhi[:].rearrange("p (h c) -> p h c", h=2)
            zlo3 = zlo[:].rearrange("p (h c) -> p h c", h=2)
            # both G halves live in one PSUM bank as a single accumulation
            # group: [G(b1) cols 0:256 | G(b2) cols 256:512]; the spare
            # 128-col block of each half later holds that b's column-sum
            gt = pg.tile([128, 512], f32, tag="gt", name=f"gt{pair}")
            gts[pair] = gt
            n_mm = 0
            for c in range(8):
                rhs_hi = zhi3[:, :, c * 128:(c + 1) * 128]
                rhs_lo = zlo3[:, :, c * 128:(c + 1) * 128]
                for h in (0, 1):
                    lo = h * 1024 + c * 128
                    for lh, rh in (
                        (zhi[:, lo:lo + 128], rhs_hi),
                        (zhi[:, lo:lo + 128], rhs_lo),
                        (zlo[:, lo:lo + 128], rhs_hi),
                    ):
                        nc.tensor.matmul(
                            gt[:, h * 256:(h + 1) * 256],
                            lh,
                            rh,
                            start=(n_mm == 0),
                            stop=(n_mm == 47),
                        )
                        n_mm += 1

        def emit_diag_start(pair):
            # G -> DRAM -> stride-129 diagonal reload -> signed mask; no PE
            # work, so it can chase the grams immediately
            gt = gts[pair]
            for h, b in ((0, 2 * pair), (1, 2 * pair + 1)):
                gd = g_dram[b]
                gsb = g_pool.tile([128, 128], f32, tag="g", name=f"gsb{b}")
                nc.scalar.copy(gsb[:], gt[:, h * 256 + h * 128:h * 256 + (h + 1) * 128])
                nc.sync.dma_start(gd.ap()[0:128, :], gsb[:])
                upt = up_pool.tile([128, 128], f32, tag="up", name=f"up{b}")
                upts[b] = upt
                nc.sync.dma_start(upt[:], AP(gd, 0, [[129, 128], [1, 128]]))
                nc.sync.dma_start(
                    upt[64:128, 64:128], AP(gd, 64 * 128, [[129, 64], [1, 64]])
                )
                nc.vector.tensor_mul(upt[:], upt[:], msk_t[:])

        def emit_diag_finish(pair):
            # column-sums (PE) one pipeline stage later, when the DRAM
            # round-trip has surely completed
            gt = gts.pop(pair)
            for h, b in ((0, 2 * pair), (1, 2 * pair + 1)):
                upt = upts.pop(b)
                wcol = gt[:, h * 256 + (1 - h) * 128:h * 256 + (1 - h) * 128 + 1]
                nc.tensor.matmul(wcol, upt[:], ones_t[:])
                nc.scalar.copy(W_ts[b // 8][:, (b % 8):(b % 8) + 1], wcol)

        # software pipeline: transposes of pair p run on the PE while the
        # grams of pair p-1 stream and the diagonal extraction of pair p-2
        # finishes, hiding both the PSUM-evacuation (copy+subtract) latency
        # and the DRAM diagonal round-trip
        def emit_body(inline_ss, first_xpp=None):
            zq_prev = emit_transposes(0, first_xpp or load_pair_inputs(0))
            sp4_lo = None
            for pair in range(1, BL // 2):
                xpp = load_pair_inputs(pair)
                zq = emit_transposes(pair, xpp)
                emit_grams(pair - 1, *zq_prev)
                emit_diag_start(pair - 1)
                if pair >= 2:
                    emit_diag_finish(pair - 2)
                zq_prev = zq
                if inline_ss and pair == 3:
                    emit_ss_load()
                if inline_ss and pair == 4:
                    emit_ss_prep()
                if pair - 2 == 3:
                    sp4_lo = do_spectrum_half(0)
                if pair - 2 == 4:
                    do_peaks_half(0, sp4_lo)
            last = BL // 2 - 1
            emit_grams(last, *zq_prev)
            emit_diag_start(last)
            emit_diag_finish(last - 1)
            emit_diag_finish(last)
            sp4_hi = do_spectrum_half(1)
            do_peaks_half(1, sp4_hi)

        if loop_n is None:
            emit_body(inline_ss=True, first_xpp=xpp_next)
        else:
            emit_ss_load()
            emit_ss_prep()
            with tc.For_i(0, loop_n, 1):
                emit_body(inline_ss=False)

    nc.compile()
    _cache[key] = nc
    return nc


def _is_ula(sr, si, atol=1e-3):
    """Check the steering matrix has the phase-additive ULA structure the
    Toeplitz reduction relies on."""
    if not (np.allclose(sr[0], 1.0, atol=atol) and np.allclose(si[0], 0.0, atol=atol)):
        return False
    # conj(S_n) * S_{n+1} should equal S_1 for every n
    re = sr[:-1] * sr[1:] + si[:-1] * si[1:]
    im = sr[:-1] * si[1:] - si[:-1] * sr[1:]
    return bool(
        np.allclose(re, sr[1][None, :], atol=atol)
        and np.allclose(im, si[1][None, :], atol=atol)
    )


def _fallback_numpy(x_real, x_imag, steer_real, steer_imag, angles, M):
    x = x_real.astype(np.float32) + 1j * x_imag.astype(np.float32)
    cov = np.matmul(x, np.conj(np.swapaxes(x, 1, 2))) / np.float32(T)
    S = steer_real.astype(np.float32) + 1j * steer_imag.astype(np.float32)
    spec = np.einsum("na,bnm,ma->ba", np.conj(S), cov, S).real.astype(np.float32)
    labels = np.zeros((spec.shape[0], M), np.float32)
    for b in range(spec.shape[0]):
        s = spec[b]
        pk = (s[1:-1] > s[:-2]) & (s[1:-1] > s[2:])
        masked = np.full(A, -np.inf, np.float32)
        masked[1:-1][pk] = s[1:-1][pk]
        order = np.argsort(-masked, kind="stable")[:M]
        labels[b] = angles[order]
    return labels


def _select_labels(spec, idx8, val8, x_real, x_imag, steer_real, steer_imag,
                   angles, M):
    """Top-M selection from device top-8 candidates with fp64 refinement of
    numerically risky (near-tie / flat-top) cases."""
    S64 = steer_real.astype(np.float64) + 1j * steer_imag.astype(np.float64)
    labels = np.zeros((B, M), np.float32)
    for b in range(B):
        cands = idx8[b].astype(np.int64)
        vals = val8[b].astype(np.float64)
        s = spec[b]
        suspect = np.zeros(8, bool)
        for j in range(7):
            if vals[j + 1] > -1e29 and (vals[j] - vals[j + 1]) < RISK_RANK * abs(vals[j]):
                suspect[j] = suspect[j + 1] = True
        flat = np.zeros(8, bool)
        for j, c in enumerate(cands):
            if 1 <= c <= A - 2 and (
                abs(s[c] - s[c - 1]) < RISK_FLAT * abs(s[c])
                or abs(s[c] - s[c + 1]) < RISK_FLAT * abs(s[c])
            ):
                suspect[j] = flat[j] = True
        if not suspect.any():
            labels[b] = angles[cands[:M]]
            continue
        # fp64 evaluation of the reference's direct quadratic form at the
        # union of suspect windows
        bins = set()
        for j in range(8):
            if flat[j]:
                for o in range(-3, 4):
                    if 0 <= cands[j] + o < A:
                        bins.add(int(cands[j] + o))
            elif suspect[j]:
                bins.add(int(cands[j]))
        bins = sorted(bins)
        x64 = x_real[b].astype(np.float64) + 1j * x_imag[b].astype(np.float64)
        Y = np.conj(x64).T @ S64[:, bins]  # (T, len(bins))
        sv = dict(zip(bins, (np.abs(Y) ** 2).sum(axis=0) / T))
        # refined candidate list: (value, device_rank, position)
        refined = []
        for j in range(8):
            c = int(cands[j])
            if vals[j] < -1e29:
                continue
            if flat[j]:
                # true local-max position near c per fp64
                best = None
                for o in range(-2, 3):
                    a = c + o
                    if a - 1 in sv and a + 1 in sv and a in sv:
                        if sv[a] > sv[a - 1] and sv[a] > sv[a + 1]:
                            if best is None or sv[a] > sv[best]:
                                best = a
                if best is None:
                    best = c
                refined.append((float(sv[best]), best))
            elif suspect[j]:
                refined.append((float(sv[c]), c))
            else:
                refined.append((float(vals[j]), c))
        # dedupe positions (two flat candidates can refine to the same bin)
        seen = {}
        for v, p in refined:
            if p not in seen or v > seen[p]:
                seen[p] = v
        order = sorted(seen.items(), key=lambda kv: (-kv[1], kv[0]))
        sel = [p for p, _ in order[:M]]
        while len(sel) < M:
            for c in cands:
                if int(c) not in sel:
                    sel.append(int(c))
                    break
        labels[b] = angles[sel]
    return labels


def kernel(x_real, x_imag, steer_real, steer_imag, angles, M):
    x_real = np.ascontiguousarray(np.asarray(x_real), dtype=np.float32)
    x_imag = np.ascontiguousarray(np.asarray(x_imag), dtype=np.float32)
    steer_real = np.ascontiguousarray(np.asarray(steer_real), dtype=np.float32)
    steer_imag = np.ascontiguousarray(np.asarray(steer_imag), dtype=np.float32)
    angles = np.asarray(angles)
    M = int(M)

    if (
        x_real.shape != (B, N, T)
        or steer_real.shape != (N, A)
        or M > 8
        or not _is_ula(steer_real, steer_imag)
    ):
        return _fallback_numpy(x_real, x_imag, steer_real, steer_imag, angles, M)

    from concourse.bass_utils import run_bass_kernel_spmd

    nc = build_program()
    ident, masksgn, onescol, scalevec = _host_constants()
    in_maps = []
    for c in range(NCORES):
        sl = slice(c * BL, (c + 1) * BL)
        in_maps.append({
            "xr": x_real[sl],
            "xi": x_imag[sl],
            "sr": steer_real,
            "si": steer_imag,
            "ident": ident,
            "masksgn": masksgn,
            "onescol": onescol,
            "scalevec": scalevec,
        })
    res = run_bass_kernel_spmd(nc, in_maps, list(range(NCORES))).results

    spec = np.concatenate([res[c]["out_spec"] for c in range(NCORES)], axis=0)
    idx8, val8 = _merge_quarters(
        [res[c]["out_idx"] for c in range(NCORES)],
        [res[c]["out_val"] for c in range(NCORES)],
    )

    return _select_labels(
        spec, idx8, val8, x_real, x_imag, steer_real, steer_imag, angles, M
    )


def _merge_quarters(idx_list, val_list):
    """Merge per-(eighth-chunk, batch) top-8 rows into per-batch global
    top-8. Device row r of a core: half h = r // 64, chunk j = (r % 64) // 8,
    local batch = 8*h + (r % 8); local column l maps to global angle index
    450*j - 1 + l. Non-peak filler entries are 0.0."""
    ncores = len(idx_list)
    idx8 = np.zeros((ncores * BL, 8), np.int64)
    val8 = np.full((ncores * BL, 8), -np.float64(BIG), np.float32)
    for c in range(ncores):
        iv = idx_list[c].astype(np.int64)  # (8*BL, 8)
        vv = val_list[c]
        for bl in range(BL):
            h, bi = bl // 8, bl % 8
            cand_v = []
            cand_i = []
            for j in range(8):
                r = 64 * h + 8 * j + bi
                gi = 450 * j - 1 + iv[r]
                keep = vv[r] > 0.5
                cand_v.append(vv[r][keep])
                cand_i.append(gi[keep])
            cv = np.concatenate(cand_v)
            ci = np.concatenate(cand_i)
            order = np.lexsort((ci, -cv.astype(np.float64)))[:8]
            b = c * BL + bl
            val8[b, :len(order)] = cv[order]
            idx8[b, :len(order)] = ci[order]
    return idx8, val8
